# revision 78
# baseline (speedup 1.0000x reference)
"""BartAttention (focused-attention variant) Trainium2 Bass kernel.

Problem (hardcoded): B=2, T=2048, D=1024, H=16 heads, hd=64.
  q = (h @ Wq.T + bq) * hd**-0.5 ; k = h @ Wk.T + bk ; v = h @ Wv.T + bv
  scores = q @ k.T per head ; e = f * exp(scores) ; attn = e / rowsum(e)
  out = (attn @ v) @ Wo.T + bo

Sharding over 8 cores: batch (2) x head-group (4 groups of 4 heads).
Each core computes its heads' QKV, attention, and a partial out-projection
(contraction over its 256 d-columns of Wo); host sums the 4 bf16 partials
per batch in f32 and adds bo.

On-device layout (per core):
  hT   [1024, 2048] bf16   hidden.T               (c on partitions)
  qT,kT [256, 2048] bf16   q.T / k.T              (head*hd on partitions)
  v    [2048, 4, 65] bf16  v per head + ones col  (s on partitions)
  scores computed transposed: sT[s,t] = k @ q.T so that e=f.T*exp(sT) has
  s on partitions, which is the contraction dim of the PV matmul.
  PV: outT_aug[65, t] = [v | 1].T @ e  -> row 64 = rowsum(e) per t (exact fp32).
  out-proj: final[t, m] = outT.T @ Wo_slice.T, scaled per head by 1/rowsum
  (1/rowsum broadcast across partitions by GPSIMD, multiply on DVE at 2x,
  reading the raw PV accumulator straight from PSUM - no staging copy).
  The v-bias never touches the device: attention rows sum to exactly 1,
  so attn @ (v + bv) = attn @ v + bv, and the host folds bv @ Wo.T into
  the output bias.

The 8 head-pair units (tch, j) split into a "front" (scores -> exp ->
fused broadcast f-mul, needs only the 2 `sc` PSUM tiles; units 1-6 stash
the result in one of two SBUF e-stashes, u0/u7 cycle through the 5-deep
e-tile ring) and a "back" (16 PV accumulation steps from the stash,
needs only 2 `pv` PSUM banks).  Splitting fronts from backs decouples
the exp stream from PSUM accumulator residency, which is what allows
~48 of the 128 exps to hide inside the PE-bound QKV chunk phase.

Emission is driven by a virtual-clock list scheduler: per-engine virtual
times (PE/ACT/DVE/Pool) advance as instructions are emitted, a front step
is emitted only when ACT would otherwise run dry AND an `sc` slot will be
free when its scores reach the PE queue head, and otherwise
ACT-independent PE filler is emitted: QKV half-chains during the first
quarter, then PV back-steps and out-projection bursts (merged to one
[128,1024] staging tile + one DMA per output row-tile).

Engine placement: PE matmuls ~164us (binding); ACT = exps + tail-burst
staging; DVE = q/k bias adds, v copy-out, fused f-muls, reciprocals,
po scale-muls, most out-proj staging; Pool = 1/rowsum partition
broadcasts; weight/fT/hT DMAs split across the SP and ACT HWDGE queues.
"""

import numpy as np
import ml_dtypes

import concourse.bass as bass
import concourse.bacc as bacc
import concourse.mybir as mybir
from concourse.tile import TileContext
from concourse.bass_utils import run_bass_kernel_spmd

BF16 = mybir.dt.bfloat16
F32 = mybir.dt.float32
AF = mybir.ActivationFunctionType

B, T, D = 2, 2048, 1024
H, HD = 16, 64
HG = 4               # heads per core
R = HG * HD          # 256 d-rows per core
SCALING = HD ** -0.5
N_CORES = 8

P = 128
KT = D // P          # 8 k-tiles for QKV contraction
MT = R // P          # 2 m-tiles of qT/kT rows
NCH = T // 512       # 4 chunks of 512 along t
ST = T // P          # 16 s-tiles

E_BUFS = 5

# virtual-clock costs (ns), from the TRN2 cost model
MM512 = 512 * (1e9 / 2.4e9)          # one 512-col bf16 matmul
MM256 = 256 * (1e9 / 2.4e9)
EXP = (1024 + 222) * (1e9 / 1.2e9)   # [128,1024] exp, PSUM->SBUF
BIAS = (512 + 222) * (1e9 / 1.2e9)   # bias-add copy on ACT
FMUL = (256 + 58) * (1e9 / 0.96e9)   # [128,512] bf16 mul at DVE 2x
FMUL2 = (512 + 58) * (1e9 / 0.96e9)  # fused [128,1024] bf16 mul at DVE 2x
VCOPY = (256 + 120) * (1e9 / 0.96e9)
PRAW = (512 + 120) * (1e9 / 0.96e9)
RECIP = (512 + 120) * (1e9 / 0.96e9)
POMUL = (256 + 58) * (1e9 / 0.96e9)
OSB_DVE = (512 + 120) * (1e9 / 0.96e9)
OSB_ACT = (512 + 222) * (1e9 / 1.2e9)
BCAST = 512 * (1e9 / 1.2e9) + 95
ACT_OH = 60.0                        # dispatch/seq overheads per ACT instr
ACK = 0.0                            # pipeline-ack + sem propagation lag
LOOKAHEAD = 3000.0                   # emit front when ACT lead < this


def build_bass():
    nc = bacc.Bacc()

    hT_d = nc.declare_dram_parameter("hT", [D, T], BF16, isOutput=False)
    fT_d = nc.declare_dram_parameter("fT", [T, T], BF16, isOutput=False)
    wqT_d = nc.declare_dram_parameter("wqT", [P, MT, KT, P], BF16, isOutput=False)
    wkT_d = nc.declare_dram_parameter("wkT", [P, MT, KT, P], BF16, isOutput=False)
    wvT_d = nc.declare_dram_parameter("wvT", [D, R], BF16, isOutput=False)
    woT_d = nc.declare_dram_parameter("woT", [R, D], BF16, isOutput=False)
    bq_d = nc.declare_dram_parameter("bq", [R, 1], F32, isOutput=False)
    bk_d = nc.declare_dram_parameter("bk", [R, 1], F32, isOutput=False)
    out_d = nc.declare_dram_parameter("out_partial", [T, D], BF16, isOutput=True)

    with TileContext(nc) as tc:
        with (
            nc.allow_low_precision(reason="bf16 pipeline is intentional"),
            tc.tile_pool(name="sb", bufs=1) as sb,
            tc.tile_pool(name="ps", bufs=1, space="PSUM") as ps,
        ):
            # ---- persistent SBUF tensors ----
            hT = sb.tile([P, KT, T], BF16)
            wqT = sb.tile([P, MT, KT, P], BF16)   # [p, m, k, c] m-major
            wkT = sb.tile([P, MT, KT, P], BF16)
            wvT = sb.tile([P, KT, R], BF16)
            woT = sb.tile([P, MT, D], BF16)
            bq = sb.tile([P, MT], F32)
            bk = sb.tile([P, MT], F32)
            ones_r = sb.tile([1, P], BF16)     # K=1 lhsT for v-bias matmul
            qT = sb.tile([P, MT, T], BF16)
            kT = sb.tile([P, MT, T], BF16)
            vsb = sb.tile([P, ST, HG, HD + 1], BF16)
            po = sb.tile([P, MT, T], BF16)     # scaled outT, out-proj lhsT
            stash = [sb.tile([P, ST, 1024], BF16, name=f"stash{i}")
                     for i in range(2)]

            # warmup: a 1-column matmul as soon as ones_r is set starts the
            # PE p-state clock, so real matmuls (arriving ~4us later, past
            # the 3us ramp) run at full clock from the first chunk.
            nc.vector.memset(ones_r[:], 1.0)
            warm = ps.tile([1, 1], F32, tag="pv", bufs=4, name="warm")
            nc.tensor.matmul(warm[:], ones_r[:, 0:1], ones_r[:, 0:1],
                             start=True, stop=True)
            # dummy exp: hoists the 1.3us ACT table load to t~0 (ACT idle)
            # instead of delaying the first real exp
            warm_e = sb.tile([1, 1], BF16, name="warm_e")
            nc.scalar.activation(warm_e[:], warm[:], AF.Exp)

            # startup DMAs: first hT chunk on the SP queue; q/k weights in
            # k-halves on the ACT HWDGE queue, interleaved so the first QKV
            # matmuls get their operands as early as possible.
            hT_r = hT_d.rearrange("(k p) t -> p k t", p=P)
            wq_r = wqT_d
            wk_r = wkT_d
            nc.sync.dma_start(hT[:, 0:2, 0:512], hT_r[:, 0:2, 0:512])
            nc.scalar.dma_start(wqT[:, 0, :, :], wq_r[:, 0, :, :])
            nc.sync.dma_start(hT[:, 2:4, 0:512], hT_r[:, 2:4, 0:512])
            nc.scalar.dma_start(wkT[:, 0, :, :], wk_r[:, 0, :, :])
            nc.sync.dma_start(hT[:, 4:6, 0:512], hT_r[:, 4:6, 0:512])
            nc.sync.dma_start(hT[:, 6:8, 0:512], hT_r[:, 6:8, 0:512])
            nc.scalar.dma_start(wqT[:, 1, :, :], wq_r[:, 1, :, :])
            nc.scalar.dma_start(wkT[:, 1, :, :], wk_r[:, 1, :, :])
            nc.sync.dma_start(bq[:], bq_d.rearrange("(m p) one -> p (m one)", p=P))
            nc.sync.dma_start(bk[:], bk_d.rearrange("(m p) one -> p (m one)", p=P))
            nc.scalar.dma_start(wvT[:], wvT_d.rearrange("(k p) r -> p k r", p=P))
            nc.vector.memset(vsb[:, :, :, HD : HD + 1], 1.0)

            ft_tiles = {}
            emitted_ft = set()

            def new_ft(tch):
                ft_tiles[tch] = sb.tile([P, ST, 512], BF16,
                                        tag=f"ft{tch % 2}", bufs=1, name=f"ft_t{tch}")

            fT_r = fT_d.rearrange("(g p) c -> p g c", p=P)

            # pre-issue the first f blocks for unit u0 (needed ~9us in)
            new_ft(0)
            for blk in (0, 1):
                emitted_ft.add((0, blk))
                nc.sync.dma_start(ft_tiles[0][:, 4 * blk : 4 * blk + 4, :],
                                  fT_r[:, 4 * blk : 4 * blk + 4, 0:512])

            def ft_need(tch, st):
                """Ensure f tiles for (tch, st..st+4) are loading; loads go
                in 4-s-tile blocks to cut HWDGE dispatch serialization."""
                if tch not in ft_tiles:
                    new_ft(tch)
                for blk in {st // 4, min(st // 4 + 1, 3)}:
                    if (tch, blk) not in emitted_ft:
                        emitted_ft.add((tch, blk))
                        nc.sync.dma_start(
                            ft_tiles[tch][:, 4 * blk : 4 * blk + 4, :],
                            fT_r[:, 4 * blk : 4 * blk + 4,
                                 tch * 512 : (tch + 1) * 512],
                        )

            # ================= virtual-clock scheduler =================
            clk = {"pe": 0.0, "act": 0.0, "dve": 0.0, "pool": 0.0}

            exp_done = {}     # (u, st) -> ACT virtual completion of exp
            fmul_done = {}    # (u, st) -> DVE virtual completion of f-mul
            qk_ready = {}     # (w, n, m) -> ACT completion of bias-add
            v_ready = {}      # st -> DVE completion of v copy-out
            po_ready = {}     # u -> DVE completion of both po scale-muls
            pv_done = {}      # (u, st) -> PE completion of PV step
            sc_queue = []     # ACT completion times of in-flight exps
            e_tiles = {}      # st -> e tile for live u0
            pv_pairs = {}
            norm_hold = []    # (u, h, recip, praw, recip_done)

            def new_pv_pair(u):
                pv_pairs[u] = [ps.tile([HD + 1, 512], F32, tag="pv", bufs=4,
                                       name=f"pv_{u}_{a}") for a in range(2)]

            chain_accs = {}

            def emit_chain(w, n, m, half):
                """Half a q/k projection chain (4 k-steps): keeps PE filler
                granularity under the ACT exp turnaround so the exp stream
                never bubbles behind a long chain."""
                w_sb, b_sb, o_sb = ((wqT, bq, qT) if w == "q" else (wkT, bk, kT))
                nsl = slice(n * 512, (n + 1) * 512)
                if half == 0:
                    chain_accs[(w, n, m)] = ps.tile(
                        [P, 512], F32, tag="pv", bufs=4, name=f"{w}acc_{n}_{m}")
                acc = chain_accs[(w, n, m)]
                for k in range(4 * half, 4 * half + 4):
                    nc.tensor.matmul(
                        acc[:], w_sb[:, m, k, :], hT[:, k, nsl],
                        start=(k == 0), stop=(k == KT - 1),
                    )
                clk["pe"] += 4 * MM512
                if half == 1:
                    del chain_accs[(w, n, m)]
                    # bias-add on DVE: on ACT it would queue behind exps and
                    # delay qk_ready (the fronts' gate) at chunk edges
                    nc.vector.tensor_scalar_add(o_sb[:, m, nsl], acc[:],
                                                b_sb[:, m : m + 1])
                    clk["dve"] = max(clk["dve"], clk["pe"]) + PRAW
                    qk_ready[(w, n, m)] = clk["dve"]

            def emit_vchain(st, half):
                if half == 0:
                    chain_accs[("v", st)] = ps.tile(
                        [P, R], F32, tag="pv", bufs=4, name=f"vacc_{st}")
                acc = chain_accs[("v", st)]
                for k in range(4 * half, 4 * half + 4):
                    nc.tensor.matmul(
                        acc[:], hT[:, k, st * P : (st + 1) * P], wvT[:, k, :],
                        start=(k == 0), stop=(k == KT - 1),
                    )
                clk["pe"] += 4 * MM256
                if half == 1:
                    del chain_accs[("v", st)]
                    nc.vector.tensor_copy(
                        vsb[:, st, :, 0:HD],
                        acc[:].rearrange("p (h d) -> p h d", h=HG),
                    )
                    clk["dve"] = max(clk["dve"], clk["pe"]) + VCOPY
                    v_ready[st] = clk["dve"]

            def emit_front(u, st):
                tch, j = divmod(u, 2)
                ft_need(tch, st)
                tsl = slice(tch * 512, (tch + 1) * 512)
                ssl = slice(st * P, (st + 1) * P)
                sc = ps.tile([P, 1024], F32, tag="sc", bufs=2, name=f"sc{u}_{st}")
                for a in range(2):
                    rows = slice(a * HD, (a + 1) * HD)
                    nc.tensor.matmul(
                        sc[:, a * 512 : (a + 1) * 512],
                        kT[rows, j, ssl], qT[rows, j, tsl],
                        start=True, stop=True,
                    )
                clk["pe"] = max(clk["pe"], qk_ready[("q", tch, j)],
                                qk_ready[("k", st // 4, j)]) + 2 * MM512
                if u in (0, 7):
                    dst = sb.tile([P, 1024], BF16, tag="e", bufs=E_BUFS,
                                  name=f"e{u}_{st}")
                    e_tiles[(u, st)] = dst
                else:
                    dst = stash[u % 2][:, st, :]
                nc.scalar.activation(dst, sc[:], AF.Exp)
                war = pv_done.get((u - 2, st), 0.0) if 3 <= u <= 6 else 0.0
                clk["act"] = max(clk["act"], clk["pe"], war) + EXP + ACT_OH
                exp_done[(u, st)] = clk["act"] + ACK
                sc_queue.append(clk["act"] + ACK)
                # single fused f-mul: f broadcast across the two head halves
                fb = ft_tiles[tch][:, st : st + 1, :].to_broadcast((P, 2, 512))
                nc.vector.tensor_mul(
                    dst.rearrange("p (two c) -> p two c", two=2),
                    dst.rearrange("p (two c) -> p two c", two=2), fb)
                clk["dve"] = max(clk["dve"], clk["act"] + ACK) + FMUL2
                # consumer-side lag: cross-engine sem hops accumulate real
                # drift the virtual clocks don't carry; gate PVs later
                fmul_done[(u, st)] = clk["dve"] + ACK + 400.0

            def emit_pv(u, st):
                """PV step: u0 from its e tile, others from the stash."""
                tch, j = divmod(u, 2)
                if u in (0, 7):
                    src = e_tiles.pop((u, st))
                else:
                    src = stash[u % 2][:, st, :]
                for a in range(2):
                    nc.tensor.matmul(
                        pv_pairs[u][a][:], vsb[:, st, 2 * j + a, :],
                        src[:, a * 512 : (a + 1) * 512],
                        start=(st == 0), stop=(st == ST - 1),
                    )
                clk["pe"] = max(clk["pe"], fmul_done[(u, st)], v_ready[st])
                clk["pe"] += 2 * MM512
                pv_done[(u, st)] = clk["pe"]
                if st == ST - 1:
                    emit_norm(u)

            def emit_norm(u):
                """Whole normalize chain is deferred into flush_norms so it
                never preempts the f-mul stream on DVE at back boundaries."""
                tch, j = divmod(u, 2)
                for a in range(2):
                    norm_hold.append((u, 2 * j + a, pv_pairs[u][a],
                                      pv_done[(u, ST - 1)]))

            def flush_norms(force=False):
                while norm_hold:
                    u, h, pva, pvdone = norm_hold[0]
                    recip_t = max(clk["dve"], pvdone) + RECIP
                    pool_t = max(clk["pool"], recip_t) + BCAST
                    if not force and pool_t > clk["dve"] + RECIP + 400:
                        break
                    norm_hold.pop(0)
                    tch = u // 2
                    recip = sb.tile([1, 512], BF16, tag="recip", bufs=4,
                                    name=f"recip_{tch}_{h}")
                    nc.vector.reciprocal(recip[:], pva[HD : HD + 1, :])
                    clk["dve"] = recip_t
                    clk["pool"] = pool_t
                    bcs = sb.tile([HD, 512], BF16, tag="bcs", bufs=2,
                                  name=f"bcs_{tch}_{h}")
                    nc.gpsimd.partition_broadcast(bcs[:], recip[:])
                    nc.vector.tensor_mul(
                        po[(h % 2) * HD : (h % 2) * HD + HD, h // 2,
                           tch * 512 : (tch + 1) * 512],
                        pva[0:HD, :], bcs[:],
                    )
                    clk["dve"] = max(clk["dve"], clk["pool"]) + OSB_DVE
                    po_ready[u] = max(po_ready.get(u, 0.0), clk["dve"])

            pre_fins = {}

            def emit_prerun(tt):
                """j=0 halves of row-tile tt's out-proj: depends only on the
                second-to-last unit's po, fills the final norm-chain gap."""
                fins = []
                for n in range(2):
                    fin = ps.tile([P, 512], F32, tag="pv", bufs=4,
                                  name=f"fin_{tt}_{n}")
                    nc.tensor.matmul(
                        fin[:], po[:, 0, tt * P : (tt + 1) * P],
                        woT[:, 0, n * 512 : (n + 1) * 512],
                        start=True, stop=False,
                    )
                    clk["pe"] = max(clk["pe"], po_ready[6]) + MM512
                    fins.append(fin)
                pre_fins[tt] = fins

            def emit_burst(tt):
                """Both out-proj halves of row-tile tt, one merged DMA."""
                tch = tt // 4
                osb = sb.tile([P, 1024], BF16, tag="osb", bufs=6, name=f"osb_{tt}")
                gate = max(po_ready[2 * tch], po_ready[2 * tch + 1])
                fins0 = pre_fins.pop(tt, None)
                for n in range(2):
                    if fins0 is not None:
                        fin = fins0[n]
                        nc.tensor.matmul(
                            fin[:], po[:, 1, tt * P : (tt + 1) * P],
                            woT[:, 1, n * 512 : (n + 1) * 512],
                            start=False, stop=True,
                        )
                        clk["pe"] = max(clk["pe"], gate) + MM512
                    else:
                        fin = ps.tile([P, 512], F32, tag="pv", bufs=4,
                                      name=f"fin_{tt}_{n}")
                        for j in range(MT):
                            nc.tensor.matmul(
                                fin[:], po[:, j, tt * P : (tt + 1) * P],
                                woT[:, j, n * 512 : (n + 1) * 512],
                                start=(j == 0), stop=(j == MT - 1),
                            )
                        clk["pe"] = max(clk["pe"], gate) + 2 * MM512
                    # ACT is nearly co-critical (exp stream): keep staging
                    # copies off it until the exps are exhausted
                    use_act = (n == 0) and fi >= len(front_list)
                    if use_act:
                        nc.scalar.copy(osb[:, n * 512 : (n + 1) * 512], fin[:])
                        clk["act"] = max(clk["act"], clk["pe"]) + OSB_ACT + ACT_OH
                    else:
                        nc.vector.tensor_copy(osb[:, n * 512 : (n + 1) * 512],
                                              fin[:])
                        clk["dve"] = max(clk["dve"], clk["pe"]) + OSB_DVE
                nc.sync.dma_start(
                    out_d[tt * P : (tt + 1) * P, :], osb[:]
                )

            # ---------- work lists ----------
            chain_list = []
            for n in range(NCH):
                for m in range(MT):
                    if n == 0:
                        # chunk 0 is DMA-paced: interleave q/k halves so the
                        # k half (whose m-major weights land early) can run
                        # while the q half waits on later hT pairs
                        chain_list.append(("q", n, m, 0))
                        chain_list.append(("k", n, m, 0))
                        chain_list.append(("q", n, m, 1))
                        chain_list.append(("k", n, m, 1))
                    else:
                        chain_list.append(("q", n, m, 0))
                        chain_list.append(("q", n, m, 1))
                        chain_list.append(("k", n, m, 0))
                        chain_list.append(("k", n, m, 1))
                for st in range(4 * n, 4 * n + 4):
                    if st < 12:
                        chain_list.append(("v", st, 0))
                        chain_list.append(("v", st, 1))
            for st in range(12, ST):  # deferred into post-P0 PE valleys
                chain_list.append(("v", st, 0))
                chain_list.append(("v", st, 1))

            front_list = []
            for n in range(NCH):
                for st in range(4 * n, 4 * n + 4):
                    front_list.append((0, st))
                    front_list.append((1, st))
                if n >= 1:
                    for st in range(4 * (n - 1), 4 * n):
                        front_list.append((2, st))
            for st in range(12, ST):
                front_list.append((2, st))
            for u in range(3, 8):
                for st in range(ST):
                    front_list.append((u, st))

            # PV steps: u0 in P0 (live), then u1..u7 sequentially.
            pv_list = [(0, st) for st in range(ST)]
            pv_list += [(u, st) for u in range(1, 8) for st in range(ST)]
            burst_list = list(range(16))  # one item per out row-tile tt

            ci = fi = bi = gi = 0
            emitted_wo = False

            def front_ok(strict):
                if fi >= len(front_list):
                    return False
                u, st = front_list[fi]
                tch, j = divmod(u, 2)
                if ("q", tch, j) not in qk_ready:
                    return False
                if ("k", st // 4, j) not in qk_ready:
                    return False
                if 3 <= u <= 6 and (u - 2, st) not in pv_done:
                    return False  # stash WAR: hard correctness dependency
                if not strict:
                    return True
                pend = sum(1 for t in sc_queue[-2:] if t > clk["pe"])
                return pend < 2

            def pv_ok(strict):
                if bi >= len(pv_list):
                    return False
                u, st = pv_list[bi]
                if (u, st) not in fmul_done or st not in v_ready:
                    return False
                if u == 1 and ci < len(chain_list):
                    return False  # PSUM: chains still rotating pv slots
                if u >= 1 and (u - 1 if u > 1 else 0, ST - 1) not in pv_done:
                    return False  # one stash-back pair at a time
                if not strict:
                    return True
                return fmul_done[(u, st)] <= clk["pe"] + 300

            EAGER_BURSTS = 6  # rest reserved to fill endgame ACT-paced gaps

            def burst_ok(strict):
                if gi >= len(burst_list) or not emitted_wo:
                    return False
                tch = burst_list[gi] // 4
                if 2 * tch not in po_ready or 2 * tch + 1 not in po_ready:
                    return False
                if not strict:
                    return True
                if gi >= EAGER_BURSTS and fi < len(front_list):
                    return False
                return max(po_ready[2 * tch], po_ready[2 * tch + 1]) \
                    <= clk["pe"] + 300

            new_pv_pair(0)

            import collections, os
            _branch = collections.Counter()

            while (ci < len(chain_list) or fi < len(front_list)
                   or bi < len(pv_list) or gi < len(burst_list) or norm_hold):
                flush_norms()

                _branch['iter'] += 1
                endgame = fi >= len(front_list) - 32
                if (endgame or clk["act"] - clk["pe"] < LOOKAHEAD) \
                        and front_ok(True):
                    _branch['front'] += 1
                    u, st = front_list[fi]
                    fi += 1
                    emit_front(u, st)
                    continue
                if pv_ok(True):
                    _branch['pv'] += 1
                    u, st = pv_list[bi]
                    bi += 1
                    if u not in pv_pairs:
                        new_pv_pair(u)
                    emit_pv(u, st)
                    continue
                if ci < len(chain_list):
                    _branch['chain'] += 1
                    item = chain_list[ci]
                    ci += 1
                    if item[0] == "v":
                        emit_vchain(item[1], item[2])
                    else:
                        w, n, m, half = item
                        if w == "q" and m == 0 and half == 0 and n + 1 < NCH:
                            psl = slice((n + 1) * 512, (n + 2) * 512)
                            for kk in range(0, KT, 4):
                                nc.sync.dma_start(
                                    hT[:, kk : kk + 4, psl],
                                    hT_r[:, kk : kk + 4, psl])
                        emit_chain(w, n, m, half)
                    if ci == len(chain_list):
                        nc.sync.dma_start(
                            woT[:], woT_d.rearrange("(m p) d -> p m d", p=P))
                        emitted_wo = True
                    continue
                if burst_ok(True):
                    _branch['burst'] += 1
                    emit_burst(burst_list[gi])
                    gi += 1
                    continue
                if (6 in po_ready and 7 not in po_ready
                        and bi >= len(pv_list) and gi == 12
                        and 12 not in pre_fins):
                    emit_prerun(12)
                    continue

                # nothing cleanly ready: a reserved burst never stalls,
                # then the least-stalling of PV/front
                if burst_ok(False) and \
                        max(po_ready[2 * (burst_list[gi] // 4)],
                            po_ready[2 * (burst_list[gi] // 4) + 1]) \
                        <= clk["pe"] + 300:
                    _branch['fb_burst'] += 1
                    emit_burst(burst_list[gi])
                    gi += 1
                elif pv_ok(False):
                    _branch['fb_pv'] += 1
                    u, st = pv_list[bi]
                    bi += 1
                    if u not in pv_pairs:
                        new_pv_pair(u)
                    emit_pv(u, st)
                elif burst_ok(False):
                    emit_burst(burst_list[gi])
                    gi += 1
                elif front_ok(False):
                    _branch['fb_front'] += 1
                    u, st = front_list[fi]
                    fi += 1
                    emit_front(u, st)
                elif norm_hold:
                    flush_norms(force=True)
                else:
                    raise RuntimeError(
                        f"scheduler wedged: ci={ci} fi={fi} bi={bi} gi={gi}")

            if os.environ.get('SCHED_DEBUG'):
                print('branches:', dict(_branch))
    return nc


_NC = None
_LAST_RESULT = None


def _get_nc():
    global _NC
    if _NC is None:
        _NC = build_bass()
        if not _NC.is_finalized():
            _NC.finalize()
    return _NC


def _mmajor(wT):
    """[D, R] -> [p, m, k, c] so each m-tile's weights are one contiguous
    2KB run per partition (fast, early startup DMA)."""
    a = wT.reshape(KT, P, MT, P).transpose(1, 2, 0, 3)
    return np.ascontiguousarray(a)


def kernel(hidden_states, focused_attention, Wq, bq, Wk, bk, Wv, bv, Wo, bo):
    bf = ml_dtypes.bfloat16
    hT = [np.ascontiguousarray(hidden_states[b].T).astype(bf) for b in range(B)]
    fT = [np.ascontiguousarray(focused_attention[b].T).astype(bf) for b in range(B)]

    in_maps = []
    for c in range(N_CORES):
        b, g = divmod(c, 4)
        rows = slice(g * R, (g + 1) * R)
        in_maps.append({
            "hT": hT[b],
            "fT": fT[b],
            "wqT": _mmajor((Wq[rows] * SCALING).T.astype(bf)),
            "wkT": _mmajor(Wk[rows].T.astype(bf)),
            "wvT": np.ascontiguousarray(Wv[rows].T).astype(bf),
            "woT": np.ascontiguousarray(Wo[:, rows].T).astype(bf),
            "bq": np.ascontiguousarray((bq[rows] * SCALING)[:, None]).astype(np.float32),
            "bk": np.ascontiguousarray(bk[rows][:, None]).astype(np.float32),
        })

    res = run_bass_kernel_spmd(_get_nc(), in_maps, list(range(N_CORES)))
    global _LAST_RESULT
    _LAST_RESULT = res
    out = np.zeros((B, T, D), dtype=np.float32)
    for c in range(N_CORES):
        out[c // 4] += np.asarray(res.results[c]["out_partial"], dtype=np.float32)
    # attn rows sum to 1, so the v-bias contributes exactly bv per row:
    # fold it into the output constant instead of computing it on-device
    bo_eff = np.asarray(bo, dtype=np.float32) + \
        np.asarray(bv, dtype=np.float32) @ np.asarray(Wo, dtype=np.float32).T
    out += bo_eff[None, None, :]
    return out
